# revision 5
# baseline (speedup 1.0000x reference)
"""Cross-attention fusion kernel for Trainium2 (8 NeuronCores).

Reference computation (per sample b):
    q = Wq @ xs + bq            xs = x_s2[b] as [256, 4096]
    k = Wk @ xd + bk            xd = x_dem[b] as [64, 4096]
    v = Wv @ xd + bv
    attn = softmax_j(k^T q * c)             c = 256 ** -0.5
    out = v @ attn + x_s2[b]                out[ch, j] = sum_i v[ch, i] attn[i, j]

Device-side restructure (mathematically identical):
  - logits = k^T q * c = (M^T xd_aug)^T xs with M = [Wk^T; bk] @ (Wq * c)
    precomputed on the host ([65, 256]); neither q nor k materializes.
  - bq adds a per-i constant to logits, which cancels in softmax_j -> dropped.
  - bk / bv folded in via a ones row appended to xd (contraction K=65).
  - softmax denominators folded into v columns (scale v[:, i] by 1/sum_j e).
  - exp without running-max shift: logits are O(1); the fp8 e-matrix is
    range-shifted by a fixed -ln(4).
  - BOTH big matmuls run fp8 DoubleRow (K=256/instr, 2 MACs/cell/cycle):
    phase D contracts kq8 (x64) against xs8 (x16, quantized on host), the
    exp ACTIVATE un-scales via its free affine (scale=1/1024); phase E
    contracts vts (fp8) against e (fp8).
  - The exp stream runs at FD=1024 from a 4-bank PSUM double buffer so the
    other 4 banks hold interleaved phase-E accumulation chains: each output
    chunk accumulates as 3 PSUM-resident bursts (i-pairs 0-3 / 4-6 / 7)
    merged through SBUF by the DVE while the ACT engine (the bottleneck,
    ~82us of exp+accum) streams uninterrupted.
  - 1/ALPHA_V and the residual add happen on the host.

Sharding: 8 cores = 4 samples x 2 halves of the key-pixel axis i. Each core
emits a partial out [256, 4096] * ALPHA_V; the host sums the two halves,
divides, and adds the residual. No collectives.
"""

import numpy as np
import ml_dtypes

import concourse.bass as bass
import concourse.mybir as mybir
import concourse.tile as tile
from concourse import bacc
from concourse.bass_utils import run_bass_kernel_spmd

P = 128
CH = 256          # out_ch == s2_ch
DEM = 64          # dem_ch
N = 4096          # pixels per sample (j axis)
NI = 2048         # key pixels per core (i axis, half of N)
KO = CH // P      # 2 partition chunks of the 256-channel axis
NIB = NI // P     # 16 i-blocks per core
NPAIR = NIB // 2  # 8 i-block pairs (DoubleRow K=256 units)
NCORES = 8

F32 = mybir.dt.float32
BF16 = mybir.dt.bfloat16
FP8 = mybir.dt.float8e4
NP_BF16 = ml_dtypes.bfloat16
NP_FP8 = ml_dtypes.float8_e4m3

SXS = 16.0        # host scale for xs fp8
SKQ = 64.0        # device scale for kq fp8
ALPHA_V = 8192.0  # vts fp8 scale; undone on the host
E_BIAS = -1.3862943611198906  # -ln(4)

DR = mybir.MatmulPerfMode.DoubleRow


def build_bass():
    nc = bacc.Bacc(None, target_bir_lowering=False)

    xs8_d = nc.dram_tensor("xs8", [CH, N], FP8, kind="ExternalInput")
    xda_d = nc.dram_tensor("xda", [DEM + 1, NI], BF16, kind="ExternalInput")
    wm_d = nc.dram_tensor("wm", [DEM + 1, CH], BF16, kind="ExternalInput")
    wv_d = nc.dram_tensor("wv", [DEM + 1, CH], BF16, kind="ExternalInput")
    out_d = nc.dram_tensor("out", [CH, N], BF16, kind="ExternalOutput")

    xs8_v = xs8_d.ap().rearrange("(ko p) j -> p ko j", p=P)
    out_v = out_d.ap().rearrange("(m p) j -> p m j", p=P)

    with tile.TileContext(nc) as tc:
        with (
            tc.tile_pool(name="consts", bufs=1) as consts,
            tc.tile_pool(name="bigs", bufs=1) as bigs,
            tc.tile_pool(name="small", bufs=1) as small,
        ):
            # ---- SBUF tiles ----
            xda_sb = consts.tile([DEM + 1, NI], BF16)
            wm_sb = consts.tile([DEM + 1, CH], BF16)
            wv_sb = consts.tile([DEM + 1, CH], BF16)
            xs8_sb = bigs.tile([P, KO, N], FP8)
            kq8_sb = bigs.tile([P, KO, NI], FP8)     # kq * 64, ci via (p, ko)
            vt_sb = bigs.tile([P, NIB, CH], BF16)    # v^T[i, ch]
            vts_sb = bigs.tile([P, NIB, CH], FP8)    # v^T * r * ALPHA_V
            e_sb = bigs.tile([P, NIB, N], FP8)       # exp(z - ln4)
            pbuf = bigs.tile([P, KO * 8, 512], BF16)  # E chain partials
            estage = bigs.tile([P, KO, N], BF16)     # out staging (x ALPHA_V)

            sums_sb = small.tile([P, NIB, 4], F32)
            r_sb = small.tile([P, NIB], F32)
            ebias_sb = small.tile([P, 1], F32)
            dumm_sb = small.tile([P, 16], BF16)
            dummo_sb = small.tile([P, 16], BF16)
            warm_sb = small.tile([P, 512], BF16)
            nc.vector.memset(ebias_sb, E_BIAS)
            nc.vector.memset(dumm_sb, 0.0)
            nc.vector.memset(warm_sb, 0.0)
            # ACT table prefetch: tiny exp while DMAs are in flight.
            nc.scalar.activation(
                out=dummo_sb, in_=dumm_sb,
                func=mybir.ActivationFunctionType.Exp, bias=ebias_sb,
            )

            # ---- input DMAs, ordered by first use ----
            nc.sync.dma_start(out=wm_sb, in_=wm_d.ap())
            nc.sync.dma_start(out=xda_sb, in_=xda_d.ap())
            for jh in range(2):
                nc.sync.dma_start(
                    out=xs8_sb[:, :, jh * 2048:(jh + 1) * 2048],
                    in_=xs8_v[:, :, jh * 2048:(jh + 1) * 2048],
                )
            nc.sync.dma_start(out=wv_sb, in_=wv_d.ap())

            with (
                tc.tile_pool(name="dpsum", bufs=2, space="PSUM") as dpsum,
                tc.tile_pool(name="epsum", bufs=4, space="PSUM") as epsum,
            ):
                # Warm the PE's HAM clock gate during the input DMAs.
                for w in range(6):
                    wp = epsum.tile([P, 512], F32, tag="ep")
                    nc.tensor.matmul(
                        wp, lhsT=warm_sb[:, :P], rhs=warm_sb,
                        start=True, stop=True,
                    )

                # ---- Phase A: kq8 = 64 * M^T xda, fp8, via [128,1024] chunks
                # evictions alternate DVE/ACT so neither serializes the start.
                for ic in range(2):
                    for ko in range(KO):
                        ap_ = dpsum.tile([P, 1024], F32, tag="dp")
                        for q in range(2):
                            nc.tensor.matmul(
                                ap_[:, q * 512:(q + 1) * 512],
                                lhsT=wm_sb[:, ko * P:(ko + 1) * P],
                                rhs=xda_sb[:, ic * 1024 + q * 512:
                                           ic * 1024 + (q + 1) * 512],
                                start=True, stop=True,
                            )
                        dst = kq8_sb[:, ko, ic * 1024:(ic + 1) * 1024]
                        if ko == 0:
                            nc.vector.tensor_scalar_mul(
                                out=dst, in0=ap_, scalar1=SKQ)
                        else:
                            nc.scalar.mul(out=dst, in_=ap_, mul=SKQ)

                # ---- Phase B: v^T = [xd; 1]^T @ [Wv^T; bv] ----
                for g in range(4):
                    bp = dpsum.tile([P, 1024], F32, tag="dp")
                    for q in range(4):
                        ib = g * 4 + q
                        nc.tensor.matmul(
                            bp[:, q * 256:(q + 1) * 256],
                            lhsT=xda_sb[:, ib * P:(ib + 1) * P],
                            rhs=wv_sb,
                            start=True, stop=True,
                        )
                    bp_v = bp.rearrange("p (q s) -> p q s", s=256)
                    nc.vector.tensor_copy(
                        out=vt_sb[:, g * 4:(g + 1) * 4, :], in_=bp_v)

                # ---- interleaved phase D (logits+exp) and phase E bursts --
                # E chunk = (jc in 0..7, ko); chain over 8 i-pairs as segs
                # A=pairs 0-3, B=4-6, C=7. Burst emission schedule (by ib):
                ECHUNKS = [(jc, ko) for jc in range(8) for ko in range(KO)]
                segA_sched = {8: 3, 9: 3, 10: 3, 11: 3, 12: 2, 13: 2}
                segB_sched = {14: 8, 15: 8}
                segA_q = list(ECHUNKS)
                segB_q = list(ECHUNKS)

                def chunk_idx(jc, ko):
                    return jc * KO + ko

                def burst(jc, ko, p0, p1, seg):
                    ep = epsum.tile([P, 512], F32, tag="ep")
                    for p in range(p0, p1 + 1):
                        nc.tensor.matmul(
                            ep,
                            lhsT=vts_sb[:, 2 * p:2 * p + 2,
                                        ko * P:(ko + 1) * P],
                            rhs=e_sb[:, 2 * p:2 * p + 2,
                                     jc * 512:(jc + 1) * 512],
                            start=(p == p0), stop=(p == p1),
                            perf_mode=DR,
                        )
                    pb = pbuf[:, chunk_idx(jc, ko), :]
                    if seg == 0:
                        nc.vector.tensor_copy(out=pb, in_=ep)
                    elif seg == 1:
                        nc.vector.tensor_add(out=pb, in0=ep, in1=pb)
                    else:
                        nc.vector.tensor_add(
                            out=estage[:, ko, jc * 512:(jc + 1) * 512],
                            in0=ep, in1=pb)

                for ib in range(NIB):
                    nA = segA_sched.get(ib, 0)
                    nB = segB_sched.get(ib, 0)
                    for c in range(4):
                        dp = dpsum.tile([P, 1024], F32, tag="dp")
                        for q in range(2):
                            j0 = c * 1024 + q * 512
                            nc.tensor.matmul(
                                dp[:, q * 512:(q + 1) * 512],
                                lhsT=kq8_sb[:, :, ib * P:(ib + 1) * P],
                                rhs=xs8_sb[:, :, j0:j0 + 512],
                                start=True, stop=True,
                                perf_mode=DR,
                            )
                        nc.scalar.activation(
                            out=e_sb[:, ib, c * 1024:(c + 1) * 1024],
                            in_=dp,
                            func=mybir.ActivationFunctionType.Exp,
                            bias=ebias_sb,
                            scale=1.0 / (SXS * SKQ),
                            accum_out=sums_sb[:, ib, c:c + 1],
                        )
                        # spread this ib's E bursts across its 4 chunks
                        if nA and c < nA:
                            jc, ko = segA_q.pop(0)
                            burst(jc, ko, 0, 3, seg=0)
                        if nB and c < nB // 2:
                            jc, ko = segB_q.pop(0)
                            burst(jc, ko, 4, 6, seg=1)
                            jc, ko = segB_q.pop(0)
                            burst(jc, ko, 4, 6, seg=1)
                    if ib % 2 == 1:
                        # r = 1/sum over the pair's two rows, then vts
                        for i2 in (ib - 1, ib):
                            nc.vector.reduce_sum(
                                out=r_sb[:, i2:i2 + 1],
                                in_=sums_sb[:, i2, :],
                                axis=mybir.AxisListType.X,
                            )
                        nc.vector.reciprocal(
                            out=r_sb[:, ib - 1:ib + 1],
                            in_=r_sb[:, ib - 1:ib + 1])
                        for i2 in (ib - 1, ib):
                            nc.vector.tensor_scalar(
                                out=vts_sb[:, i2, :],
                                in0=vt_sb[:, i2, :],
                                scalar1=r_sb[:, i2:i2 + 1],
                                scalar2=ALPHA_V,
                                op0=mybir.AluOpType.mult,
                                op1=mybir.AluOpType.mult,
                            )

                # ---- tail: leftover segA/segB, then segC + DMA out ----
                for jc, ko in segA_q:
                    burst(jc, ko, 0, 3, seg=0)
                for jc, ko in segB_q:
                    burst(jc, ko, 4, 6, seg=1)
                done_j = 0
                for jc in range(8):
                    for ko in range(KO):
                        burst(jc, ko, 7, 7, seg=2)
                    # DMA out in 1024-col chunks as estage completes
                    if jc % 2 == 1:
                        j0 = (jc - 1) * 512
                        nc.sync.dma_start(
                            out=out_v[:, :, j0:j0 + 1024],
                            in_=estage[:, :, j0:j0 + 1024],
                        )
                        done_j = (jc + 1) * 512
                assert done_j == N
    nc.finalize()
    return nc


_NC_CACHE = None


def _get_nc():
    global _NC_CACHE
    if _NC_CACHE is None:
        _NC_CACHE = build_bass()
    return _NC_CACHE


def make_in_maps(x_s2, x_dem, Wq, bq, Wk, bk, Wv, bv):
    scale = np.float32(CH ** -0.5)
    wk_aug = np.concatenate([Wk.T, bk[None, :]], axis=0)          # [65, 256]
    wm = (wk_aug @ (Wq * scale)).astype(NP_BF16)                  # [65, 256]
    wv_aug = np.concatenate([Wv.T, bv[None, :]], axis=0).astype(NP_BF16)
    ones = np.ones((1, NI), np.float32)
    in_maps = []
    for c in range(NCORES):
        s, h = divmod(c, 2)
        xs8 = np.ascontiguousarray(
            x_s2[s].reshape(CH, N) * SXS).astype(NP_FP8)
        xd = x_dem[s].reshape(DEM, N)[:, h * NI:(h + 1) * NI]
        xda = np.concatenate([xd, ones], axis=0).astype(NP_BF16)
        in_maps.append({"xs8": xs8, "xda": np.ascontiguousarray(xda),
                        "wm": wm, "wv": wv_aug})
    return in_maps


def run(inputs, trace=False, trace_cores=None):
    """Run the device kernel; returns (output, BassKernelResults)."""
    x_s2 = np.asarray(inputs["x_s2"], np.float32)
    x_dem = np.asarray(inputs["x_dem"], np.float32)
    args = {k: np.asarray(inputs[k], np.float32)
            for k in ("Wq", "bq", "Wk", "bk", "Wv", "bv")}
    in_maps = make_in_maps(x_s2, x_dem, args["Wq"], args["bq"],
                           args["Wk"], args["bk"], args["Wv"], args["bv"])
    nc = _get_nc()
    res = run_bass_kernel_spmd(nc, in_maps, core_ids=list(range(NCORES)),
                               trace=trace, trace_cores=trace_cores)
    B = x_s2.shape[0]
    out = np.empty_like(x_s2)
    inv_a = np.float32(1.0 / ALPHA_V)
    for s in range(B):
        part = (res.results[2 * s]["out"].astype(np.float32)
                + res.results[2 * s + 1]["out"].astype(np.float32))
        out[s] = (part * inv_a).reshape(CH, 64, 64) + x_s2[s]
    return out, res


def kernel(**inputs):
    out, _ = run(inputs, trace=False)
    return out


# revision 6
# speedup vs baseline: 1.0505x; 1.0505x over previous
"""Cross-attention fusion kernel for Trainium2 (8 NeuronCores).

Reference computation (per sample b):
    q = Wq @ xs + bq            xs = x_s2[b] as [256, 4096]
    k = Wk @ xd + bk            xd = x_dem[b] as [64, 4096]
    v = Wv @ xd + bv
    attn = softmax_j(k^T q * c)             c = 256 ** -0.5
    out = v @ attn + x_s2[b]                out[ch, j] = sum_i v[ch, i] attn[i, j]

Device-side restructure (mathematically identical):
  - logits = k^T q * c = (M^T xd_aug)^T xs with M = [Wk^T; bk] @ (Wq * c)
    precomputed on the host ([65, 256]); neither q nor k materializes.
  - bq adds a per-i constant to logits, which cancels in softmax_j -> dropped.
  - bk / bv folded in via a ones row appended to xd (contraction K=65).
  - softmax denominators folded into v columns (scale v[:, i] by 1/sum_j e).
  - exp without running-max shift: logits are O(1); the fp8 e-matrix is
    range-shifted by a fixed -ln(4).
  - BOTH big matmuls run fp8 DoubleRow (K=256/instr, 2 MACs/cell/cycle):
    phase D contracts kq8 (x64) against xs8 (x16, quantized on host), the
    exp ACTIVATE un-scales via its free affine (scale=1/1024); phase E
    contracts vts (fp8) against e (fp8).
  - The exp stream runs at FD=1024 from a 4-bank PSUM double buffer so the
    other 4 banks hold interleaved phase-E accumulation chains: each output
    chunk accumulates as 3 PSUM-resident bursts (i-pairs 0-3 / 4-6 / 7)
    merged through SBUF by the DVE while the ACT engine (the bottleneck,
    ~82us of exp+accum) streams uninterrupted.
  - 1/ALPHA_V and the residual add happen on the host.

Sharding: 8 cores = 4 samples x 2 halves of the key-pixel axis i. Each core
emits a partial out [256, 4096] * ALPHA_V; the host sums the two halves,
divides, and adds the residual. No collectives.
"""

import numpy as np
import ml_dtypes

import concourse.bass as bass
import concourse.mybir as mybir
import concourse.tile as tile
from concourse import bacc
from concourse.bass_utils import run_bass_kernel_spmd

P = 128
CH = 256          # out_ch == s2_ch
DEM = 64          # dem_ch
N = 4096          # pixels per sample (j axis)
NI = 2048         # key pixels per core (i axis, half of N)
KO = CH // P      # 2 partition chunks of the 256-channel axis
NIB = NI // P     # 16 i-blocks per core
NPAIR = NIB // 2  # 8 i-block pairs (DoubleRow K=256 units)
NCORES = 8

F32 = mybir.dt.float32
BF16 = mybir.dt.bfloat16
FP8 = mybir.dt.float8e4
NP_BF16 = ml_dtypes.bfloat16
NP_FP8 = ml_dtypes.float8_e4m3

SXS = 16.0        # host scale for xs fp8
SKQ = 64.0        # device scale for kq fp8
ALPHA_V = 8192.0  # vts fp8 scale; undone on the host
E_BIAS = -1.3862943611198906  # -ln(4)

DR = mybir.MatmulPerfMode.DoubleRow


def build_bass():
    nc = bacc.Bacc(None, target_bir_lowering=False)

    xs8_d = nc.dram_tensor("xs8", [CH, N], FP8, kind="ExternalInput")
    xda_d = nc.dram_tensor("xda", [DEM + 1, NI], BF16, kind="ExternalInput")
    wm_d = nc.dram_tensor("wm", [DEM + 1, CH], BF16, kind="ExternalInput")
    wv_d = nc.dram_tensor("wv", [DEM + 1, CH], BF16, kind="ExternalInput")
    out_d = nc.dram_tensor("out", [CH, N], BF16, kind="ExternalOutput")

    xs8_v = xs8_d.ap().rearrange("(ko p) j -> p ko j", p=P)
    out_v = out_d.ap().rearrange("(m p) j -> p m j", p=P)

    with tile.TileContext(nc) as tc:
        with (
            tc.tile_pool(name="consts", bufs=1) as consts,
            tc.tile_pool(name="bigs", bufs=1) as bigs,
            tc.tile_pool(name="small", bufs=1) as small,
        ):
            # ---- SBUF tiles ----
            xda_sb = consts.tile([DEM + 1, NI], BF16)
            wm_sb = consts.tile([DEM + 1, CH], BF16)
            wv_sb = consts.tile([DEM + 1, CH], BF16)
            xs8_sb = bigs.tile([P, KO, N], FP8)
            kq8_sb = bigs.tile([P, KO, NI], FP8)     # kq * 64, ci via (p, ko)
            vt_sb = bigs.tile([P, NIB, CH], BF16)    # v^T[i, ch]
            vts_sb = bigs.tile([P, NIB, CH], FP8)    # v^T * r * ALPHA_V
            e_sb = bigs.tile([P, NIB, N], FP8)       # exp(z - ln4)
            pbuf = bigs.tile([P, KO * 8, 512], BF16)  # E chain partials
            estage = bigs.tile([P, KO, N], BF16)     # out staging (x ALPHA_V)

            sums_sb = small.tile([P, NIB, 4], F32)
            r_sb = small.tile([P, NIB], F32)
            ebias_sb = small.tile([P, 1], F32)
            dumm_sb = small.tile([P, 16], BF16)
            dummo_sb = small.tile([P, 16], BF16)
            warm_sb = small.tile([P, 512], BF16)
            nc.vector.memset(ebias_sb, E_BIAS)
            nc.vector.memset(dumm_sb, 0.0)
            nc.vector.memset(warm_sb, 0.0)
            # ACT table prefetch: tiny exp while DMAs are in flight.
            nc.scalar.activation(
                out=dummo_sb, in_=dumm_sb,
                func=mybir.ActivationFunctionType.Exp, bias=ebias_sb,
            )

            # ---- input DMAs, ordered by first use ----
            nc.sync.dma_start(out=xda_sb, in_=xda_d.ap())
            nc.sync.dma_start(out=wm_sb, in_=wm_d.ap())
            for jh in range(2):
                nc.sync.dma_start(
                    out=xs8_sb[:, :, jh * 2048:(jh + 1) * 2048],
                    in_=xs8_v[:, :, jh * 2048:(jh + 1) * 2048],
                )
            nc.sync.dma_start(out=wv_sb, in_=wv_d.ap())

            def exp_chunk(dp, ib, c0, width, slot):
                nc.scalar.activation(
                    out=e_sb[:, ib, c0:c0 + width],
                    in_=dp,
                    func=mybir.ActivationFunctionType.Exp,
                    bias=ebias_sb,
                    scale=1.0 / (SXS * SKQ),
                    accum_out=sums_sb[:, ib, slot:slot + 1],
                )

            def d_mms(dp, ib, j0, nmm):
                for q in range(nmm):
                    nc.tensor.matmul(
                        dp[:, q * 512:(q + 1) * 512],
                        lhsT=kq8_sb[:, :, ib * P:(ib + 1) * P],
                        rhs=xs8_sb[:, :, j0 + q * 512:j0 + (q + 1) * 512],
                        start=True, stop=True,
                        perf_mode=DR,
                    )

            def pair_norm(ib, nsum):
                # r = 1/sum over the pair's two rows, then vts (fp8)
                for i2 in (ib - 1, ib):
                    nc.vector.reduce_sum(
                        out=r_sb[:, i2:i2 + 1],
                        in_=sums_sb[:, i2, :nsum],
                        axis=mybir.AxisListType.X,
                    )
                nc.vector.reciprocal(
                    out=r_sb[:, ib - 1:ib + 1],
                    in_=r_sb[:, ib - 1:ib + 1])
                for i2 in (ib - 1, ib):
                    nc.vector.tensor_scalar(
                        out=vts_sb[:, i2, :],
                        in0=vt_sb[:, i2, :],
                        scalar1=r_sb[:, i2:i2 + 1],
                        scalar2=ALPHA_V,
                        op0=mybir.AluOpType.mult,
                        op1=mybir.AluOpType.mult,
                    )

            # ---- Phase 1: A, B, and D for ibs 0-7 at FD=2048 (8 banks) ----
            with tc.tile_pool(name="dpsum2", bufs=2, space="PSUM") as dpsum2:
                # Warm the PE's HAM clock gate during the input DMAs.
                wp = dpsum2.tile([P, 2048], F32, tag="dp2")
                for w in range(6):
                    nc.tensor.matmul(
                        wp[:, (w % 4) * 512:(w % 4) * 512 + 512],
                        lhsT=warm_sb[:, :P], rhs=warm_sb,
                        start=True, stop=True,
                    )

                # Phase A: kq8 = 64 * M^T xda; i-halves so D can start after
                # the first half; evictions alternate DVE/ACT.
                for ic in range(2):
                    ap_ = dpsum2.tile([P, 2048], F32, tag="dp2")
                    for ko in range(KO):
                        for q in range(2):
                            nc.tensor.matmul(
                                ap_[:, ko * 1024 + q * 512:
                                    ko * 1024 + (q + 1) * 512],
                                lhsT=wm_sb[:, ko * P:(ko + 1) * P],
                                rhs=xda_sb[:, ic * 1024 + q * 512:
                                           ic * 1024 + (q + 1) * 512],
                                start=True, stop=True,
                            )
                    for ko in range(KO):
                        dst = kq8_sb[:, ko, ic * 1024:(ic + 1) * 1024]
                        src = ap_[:, ko * 1024:(ko + 1) * 1024]
                        if ko == 0:
                            nc.vector.tensor_scalar_mul(
                                out=dst, in0=src, scalar1=SKQ)
                        else:
                            nc.scalar.mul(out=dst, in_=src, mul=SKQ)

                # Phase B: v^T = [xd; 1]^T @ [Wv^T; bv]
                for g in range(4):
                    bp = dpsum2.tile([P, 2048], F32, tag="dp2")
                    for q in range(4):
                        ib = g * 4 + q
                        nc.tensor.matmul(
                            bp[:, q * 512:q * 512 + CH],
                            lhsT=xda_sb[:, ib * P:(ib + 1) * P],
                            rhs=wv_sb,
                            start=True, stop=True,
                        )
                    bp_v = bp.rearrange("p (q s) -> p q s", s=512)[:, :, :CH]
                    nc.vector.tensor_copy(
                        out=vt_sb[:, g * 4:(g + 1) * 4, :], in_=bp_v)

                # D ibs 0-7: FD=2048 exp chunks, no E work exists yet
                for ib in range(8):
                    for c in range(2):
                        dp = dpsum2.tile([P, 2048], F32, tag="dp2")
                        d_mms(dp, ib, c * 2048, 4)
                        exp_chunk(dp, ib, c * 2048, 2048, c)
                    if ib % 2 == 1:
                        pair_norm(ib, nsum=2)

            # ---- Phase 2: D ibs 8-15 at FD=1024 + interleaved E bursts ----
            # E chunk = (jc in 0..7, ko); chain over 8 i-pairs as segs
            # A=pairs 0-2 (ready at ib6), B=3-5 (ready at ib12), C=6-7.
            ECHUNKS = [(jc, ko) for jc in range(8) for ko in range(KO)]
            segA_q = list(ECHUNKS)
            segB_q = list(ECHUNKS)
            with (
                tc.tile_pool(name="dpsum", bufs=2, space="PSUM") as dpsum,
                tc.tile_pool(name="epsum", bufs=4, space="PSUM") as epsum,
            ):
                def burst(jc, ko, p0, p1, seg):
                    ep = epsum.tile([P, 512], F32, tag="ep")
                    for p in range(p0, p1 + 1):
                        nc.tensor.matmul(
                            ep,
                            lhsT=vts_sb[:, 2 * p:2 * p + 2,
                                        ko * P:(ko + 1) * P],
                            rhs=e_sb[:, 2 * p:2 * p + 2,
                                     jc * 512:(jc + 1) * 512],
                            start=(p == p0), stop=(p == p1),
                            perf_mode=DR,
                        )
                    pb = pbuf[:, jc * KO + ko, :]
                    if seg == 0:
                        nc.vector.tensor_copy(out=pb, in_=ep)
                    elif seg == 1:
                        nc.vector.tensor_add(out=pb, in0=ep, in1=pb)
                    else:
                        nc.vector.tensor_add(
                            out=estage[:, ko, jc * 512:(jc + 1) * 512],
                            in0=ep, in1=pb)

                for ib in range(8, NIB):
                    for c in range(4):
                        dp = dpsum.tile([P, 1024], F32, tag="dp")
                        d_mms(dp, ib, c * 1024, 2)
                        exp_chunk(dp, ib, c * 1024, 1024, c)
                        # spread E bursts: segA during ibs 8-11, segB 12-15
                        if ib < 12:
                            jc, ko = segA_q.pop(0)
                            burst(jc, ko, 0, 2, seg=0)
                        else:
                            jc, ko = segB_q.pop(0)
                            burst(jc, ko, 3, 5, seg=1)
                    if ib % 2 == 1:
                        pair_norm(ib, nsum=4)

                # ---- tail: segC (pairs 6-7) + DMA out ----
                assert not segA_q and not segB_q
                for jc in range(8):
                    for ko in range(KO):
                        burst(jc, ko, 6, 7, seg=2)
                    # DMA out in 1024-col chunks as estage completes
                    if jc % 2 == 1:
                        j0 = (jc - 1) * 512
                        nc.sync.dma_start(
                            out=out_v[:, :, j0:j0 + 1024],
                            in_=estage[:, :, j0:j0 + 1024],
                        )
    nc.finalize()
    return nc


_NC_CACHE = None


def _get_nc():
    global _NC_CACHE
    if _NC_CACHE is None:
        _NC_CACHE = build_bass()
    return _NC_CACHE


def make_in_maps(x_s2, x_dem, Wq, bq, Wk, bk, Wv, bv):
    scale = np.float32(CH ** -0.5)
    wk_aug = np.concatenate([Wk.T, bk[None, :]], axis=0)          # [65, 256]
    wm = (wk_aug @ (Wq * scale)).astype(NP_BF16)                  # [65, 256]
    wv_aug = np.concatenate([Wv.T, bv[None, :]], axis=0).astype(NP_BF16)
    ones = np.ones((1, NI), np.float32)
    in_maps = []
    for c in range(NCORES):
        s, h = divmod(c, 2)
        xs8 = np.ascontiguousarray(
            x_s2[s].reshape(CH, N) * SXS).astype(NP_FP8)
        xd = x_dem[s].reshape(DEM, N)[:, h * NI:(h + 1) * NI]
        xda = np.concatenate([xd, ones], axis=0).astype(NP_BF16)
        in_maps.append({"xs8": xs8, "xda": np.ascontiguousarray(xda),
                        "wm": wm, "wv": wv_aug})
    return in_maps


def run(inputs, trace=False, trace_cores=None):
    """Run the device kernel; returns (output, BassKernelResults)."""
    x_s2 = np.asarray(inputs["x_s2"], np.float32)
    x_dem = np.asarray(inputs["x_dem"], np.float32)
    args = {k: np.asarray(inputs[k], np.float32)
            for k in ("Wq", "bq", "Wk", "bk", "Wv", "bv")}
    in_maps = make_in_maps(x_s2, x_dem, args["Wq"], args["bq"],
                           args["Wk"], args["bk"], args["Wv"], args["bv"])
    nc = _get_nc()
    res = run_bass_kernel_spmd(nc, in_maps, core_ids=list(range(NCORES)),
                               trace=trace, trace_cores=trace_cores)
    B = x_s2.shape[0]
    out = np.empty_like(x_s2)
    inv_a = np.float32(1.0 / ALPHA_V)
    for s in range(B):
        part = (res.results[2 * s]["out"].astype(np.float32)
                + res.results[2 * s + 1]["out"].astype(np.float32))
        out[s] = (part * inv_a).reshape(CH, 64, 64) + x_s2[s]
    return out, res


def kernel(**inputs):
    out, _ = run(inputs, trace=False)
    return out
